# revision 31
# baseline (speedup 1.0000x reference)
"""ANOVA-kernel (order 3) Trainium2 Bass kernel.

Reference computes, per batch b: sum_d e3(x[b, :, d]) where e3 is the 3rd
elementary symmetric polynomial over the F=64 fields. Newton's identities:

    e3 = (p1^3 - 3 p1 p2 + 2 p3) / 6,   p_k[b, d] = sum_f x[b, f, d]^k

so the sequential DP scan becomes power-sum reductions. Engine split, per
[128 x 4096] tile (batch on partitions, free = (d, f) with f contiguous):

  - p1 per (b, d): DVE grouped tensor_reduce over f.
  - "sin" tiles: the Scalar engine evaluates sin(x/8) with a free
    per-partition accumulate; sum sin(t x) = t P1 - t^3 P3/6 + O(t^5 P5),
    so P3 = 384 P1f - 3072 S1 (P5 truncation contributes only ~2.4e-4
    norm-rel). This moves the whole x^3 path onto ACT.
  - remaining tiles: ACT squares, DVE reduces x^2 (p2) and runs one
    fused scalar_tensor_tensor (x2 * x with per-partition accumulate).
  - small epilogue recombines; d-reductions via fused accumulates.

The input streams as fp16 (host-side cast during the layout marshal;
quantization contributes ~1.5e-3 norm-rel vs the 2e-2 tolerance). This
halves HBM traffic — the f32 stream was the stretched critical path —
and halves the DMA's SBUF write pressure, which was slowing the
compute engines' concurrent accesses.

Sharding: pure data parallel over the batch dim across 8 NeuronCores.
Each core gets 1024 batches = 8 tiles. The host pre-transposes each shard
to [bp, D, F] (layout marshaling only; all arithmetic is on-device).
"""

import numpy as np

_B, _F, _D = 8192, 64, 64
_NCORES = 8
_BP = _B // _NCORES  # batches per core
_P = 128             # partitions per tile
_FD = _F * _D        # free elems per batch

# tiles whose x^3 sum runs on the Scalar engine via one Sin pass
_SIN_TILES = 8
# tiles whose x^2 square runs on GPSIMD
_GPS_SQUARE_TILES = 1
# trailing tiles whose p2 f-reduction runs as a GPSIMD fold tree
_GPS_P2_FOLD_TILES = 5


def build_nc(bp=_BP, sin_tiles=_SIN_TILES, gps_square_tiles=_GPS_SQUARE_TILES,
             gps_p2_fold_tiles=_GPS_P2_FOLD_TILES):
    """Build the per-core Bass graph for bp batches.

    Inputs:  "x"   [bp, 64, 64] f16 in (b, d, f) layout
    Outputs: "out" [128, bp/128] f32 with out[p, t] = y[t*128 + p]
    """
    from contextlib import ExitStack

    from concourse import bacc, mybir, tile

    f32 = mybir.dt.float32
    f16 = mybir.dt.float16
    AF = mybir.ActivationFunctionType
    OP = mybir.AluOpType
    AX = mybir.AxisListType

    T = bp // _P  # tiles per core
    q = min(sin_tiles, T)
    assert bp % _P == 0
    # evenly spread the sin tiles over the sequence
    if 0 < q < T:
        step = T / q
        sin_set = {min(T - 1, int(i * step)) for i in range(q)}
        while len(sin_set) < q:
            sin_set.add(max(set(range(T)) - sin_set))
    else:
        sin_set = set(range(T)) if q == T else set()
    gps_sq_set = {0} if gps_square_tiles else set()
    _fold_pref = [k for k in (1, 2, 3, 4, 5, 6, 7, 0) if k < T]
    fold_set = set(_fold_pref[:min(gps_p2_fold_tiles, T)])

    nc = bacc.Bacc("TRN2", target_bir_lowering=False, debug=False)
    x_ext = nc.dram_tensor("x", [bp, _D, _F], f16, kind="ExternalInput").ap()
    y_ext = nc.dram_tensor("out", [_P, T], f32, kind="ExternalOutput").ap()

    with tile.TileContext(nc) as tc, ExitStack() as ctx:
        xp = ctx.enter_context(tc.tile_pool(name="x", bufs=4))
        x2p = ctx.enter_context(tc.tile_pool(name="x2", bufs=4))
        scr = ctx.enter_context(tc.tile_pool(name="scr", bufs=1))
        pers = ctx.enter_context(tc.tile_pool(name="pers", bufs=1))

        p1b = pers.tile([_P, T * _D], f32, tag="p1b")
        p2b = pers.tile([_P, T * _D], f32, tag="p2b")
        # one extra accumulator column (index T) for tile 0's second half
        s3 = pers.tile([_P, T + 1], f32, tag="s3")    # stt tiles: sum x^3
        sa1 = pers.tile([_P, T + 1], f32, tag="sa1")  # sin: sum sin(x/8)
        p1f = pers.tile([_P, T], f32, tag="p1f")     # sin: sum_d p1
        eacc = pers.tile([_P, T], f32, tag="eacc")
        out8 = pers.tile([_P, T], f32, tag="out8")
        x3scr = scr.tile([_P, _FD], f16, tag="x3scr")    # ACT sin out
        x3scr2 = scr.tile([_P, _FD], f16, tag="x3scr2")  # DVE stt out

        dq = pers.tile([_P, T], f32, tag="dq")

        xv_dram = x_ext.rearrange("(t p) d f -> t p (d f)", p=_P)

        fb = scr.tile([_P, _FD // 2], f32, tag="fb")

        # warm the Sin activation table during the initial DMA wait
        warm = pers.tile([_P, 1], f32, tag="warm")
        nc.gpsimd.memset(warm[:], 0.0)
        nc.scalar.activation(warm[:], warm[:], AF.Sin, scale=0.125)

        def gps_fold(src3, dst, nd):
            """f-reduction (64 -> 1 per d) as a GPSIMD binary fold tree.
            src3: [128, nd, 64] view; dst: [128, nd]; fb scratch."""
            h = _F // 2
            fv = fb[:, :nd * h].rearrange("p (d f) -> p d f", d=nd, f=h)
            nc.gpsimd.tensor_add(fv[:, :, :], src3[:, :, :h], src3[:, :, h:])
            while h > 2:
                qh = h // 2
                nc.gpsimd.tensor_add(fv[:, :, :qh], fv[:, :, :qh], fv[:, :, qh:h])
                h = qh
            nc.gpsimd.tensor_add(dst, fv[:, :, 0], fv[:, :, 1])

        n_full = T * _D
        r = pers.tile([_P, n_full], f32, tag="r")
        z = pers.tile([_P, n_full], f32, tag="z")

        with nc.allow_low_precision("fp16 stream; accumulators are f32"):

            def emit_epi(k0, k1):
                """eacc/p1f cols [k0,k1): batched d-recombination round."""
                a, b = k0 * _D, k1 * _D
                n = b - a
                nc.vector.scalar_tensor_tensor(
                    r[:, a:b], p1b[:, a:b], 1.0, p1b[:, a:b],
                    OP.mult, OP.mult)
                nc.vector.scalar_tensor_tensor(
                    z[:, a:b], p2b[:, a:b], 3.0, r[:, a:b],
                    OP.mult, OP.subtract)
                nc.vector.scalar_tensor_tensor(
                    r[:, a:b], p1b[:, a:b], -1.0 / 6.0, z[:, a:b],
                    OP.mult, OP.mult)
                nc.vector.reduce_sum(
                    eacc[:, k0:k1],
                    r[:, a:b].rearrange("p (t d) -> p t d", d=_D), axis=AX.X)
                nc.vector.reduce_sum(
                    p1f[:, k0:k1],
                    p1b[:, a:b].rearrange("p (t d) -> p t d", d=_D),
                    axis=AX.X)

            def emit_piece(k, xt, lo, nd, acc_col):
                """Emit compute for d-columns [lo, lo+nd) of tile k."""
                fd = nd * _F
                xs = xt[:, lo * _F:(lo + nd) * _F]
                xview = xs.rearrange("p (d f) -> p d f", d=nd, f=_F)
                d0 = k * _D + lo
                nc.vector.reduce_sum(p1b[:, d0:d0 + nd], xview, axis=AX.X)
                x2t = x2p.tile([_P, _FD], f16, tag="ut")
                if k in gps_sq_set:
                    nc.gpsimd.tensor_mul(x2t[:, :fd], xs, xs)
                elif k in (1, 7):
                    nc.vector.tensor_mul(x2t[:, :fd], xs, xs)
                else:
                    nc.scalar.activation(x2t[:, :fd], xs, AF.Square)
                x2view = x2t[:, :fd].rearrange("p (d f) -> p d f", d=nd, f=_F)
                if k in fold_set:
                    gps_fold(x2view, p2b[:, d0:d0 + nd], nd)
                else:
                    nc.vector.reduce_sum(p2b[:, d0:d0 + nd], x2view, axis=AX.X)
                cc = slice(acc_col, acc_col + 1)
                if k in sin_set:
                    nc.scalar.activation(
                        x3scr[:, :fd], xs, AF.Sin, scale=0.125,
                        accum_out=sa1[:, cc]
                    )
                else:
                    nc.vector.scalar_tensor_tensor(
                        out=x3scr2[:, :fd],
                        in0=x2t[:, :fd],
                        scalar=1.0,
                        in1=xs,
                        op0=OP.mult,
                        op1=OP.mult,
                        accum_out=s3[:, cc],
                    )

            for k in range(T):
                if k == 0:
                    # split tile 0 into two half-loads so compute starts
                    # ~2x sooner
                    h = _FD // 2
                    xta = scr.tile([_P, _FD // 2], f16, tag="xta")
                    nc.sync.dma_start(xta[:], xv_dram[0][:, :h])
                    emit_piece(0, xta, 0, _D // 2, 0)
                    xtb = scr.tile([_P, _FD // 2], f16, tag="xtb")
                    nc.sync.dma_start(xtb[:], xv_dram[0][:, h:])
                    fd = h
                    xview_b = xtb[:].rearrange("p (d f) -> p d f", d=_D // 2, f=_F)
                    nc.vector.reduce_sum(
                        p1b[:, _D // 2:_D], xview_b, axis=AX.X
                    )
                    x2tb = x2p.tile([_P, _FD], f16, tag="ut")
                    if 0 in gps_sq_set:
                        nc.gpsimd.tensor_mul(x2tb[:, :fd], xtb[:], xtb[:])
                    else:
                        nc.scalar.activation(x2tb[:, :fd], xtb[:], AF.Square)
                    x2view_b = x2tb[:, :fd].rearrange(
                        "p (d f) -> p d f", d=_D // 2, f=_F
                    )
                    nc.vector.reduce_sum(p2b[:, _D // 2:_D], x2view_b, axis=AX.X)
                    if 0 in sin_set:
                        nc.scalar.activation(
                            x3scr[:, :fd], xtb[:], AF.Sin, scale=0.125,
                            accum_out=sa1[:, T:T + 1],
                        )
                    else:
                        nc.vector.scalar_tensor_tensor(
                            out=x3scr2[:, :fd],
                            in0=x2tb[:, :fd],
                            scalar=1.0,
                            in1=xtb[:],
                            op0=OP.mult,
                            op1=OP.mult,
                            accum_out=s3[:, T:T + 1],
                        )
                    # fold the second-half accumulators into column 0
                    if 0 in sin_set:
                        nc.vector.scalar_tensor_tensor(
                            sa1[:, 0:1], sa1[:, T:T + 1], 1.0, sa1[:, 0:1],
                            OP.mult, OP.add,
                        )
                    else:
                        nc.vector.scalar_tensor_tensor(
                            s3[:, 0:1], s3[:, T:T + 1], 1.0, s3[:, 0:1],
                            OP.mult, OP.add,
                        )
                else:
                    xt = xp.tile([_P, _FD], f16, tag="xt")
                    nc.sync.dma_start(xt[:], xv_dram[k])
                    emit_piece(k, xt, 0, _D, k)

            # ---- epilogue (single batched round) ----
            emit_epi(0, T)
            if q == T:
                # all-sin fast path: 2 batched final stts over [128, T]
                nc.vector.scalar_tensor_tensor(
                    dq[:], sa1[:, :T], -1024.0, eacc[:], OP.mult, OP.add
                )
                nc.vector.scalar_tensor_tensor(
                    out8[:], p1f[:], 128.0, dq[:], OP.mult, OP.add
                )
            else:
                for k in range(T):
                    kk = slice(k, k + 1)
                    if k in sin_set:
                        nc.vector.scalar_tensor_tensor(
                            dq[:, kk], sa1[:, kk], -1024.0, eacc[:, kk],
                            OP.mult, OP.add,
                        )
                        nc.vector.scalar_tensor_tensor(
                            out8[:, kk], p1f[:, kk], 128.0, dq[:, kk],
                            OP.mult, OP.add
                        )
                    else:
                        nc.vector.scalar_tensor_tensor(
                            out8[:, kk], s3[:, kk], 1.0 / 3.0, eacc[:, kk],
                            OP.mult, OP.add,
                        )
            nc.sync.dma_start(y_ext[:], out8[:])

    nc.compile()
    return nc


_nc_cache = {}


def _get_nc():
    key = (_BP, _SIN_TILES, _GPS_SQUARE_TILES, _GPS_P2_FOLD_TILES)
    if key not in _nc_cache:
        _nc_cache[key] = build_nc(_BP, _SIN_TILES, _GPS_SQUARE_TILES,
                                  _GPS_P2_FOLD_TILES)
    return _nc_cache[key]


def _make_in_maps(x: np.ndarray) -> list:
    """Shard + transpose to [bp, D, F] and cast to fp16 (host marshaling)."""
    xt = np.ascontiguousarray(
        x.reshape(_NCORES, _BP, _F, _D).transpose(0, 1, 3, 2)
    ).astype(np.float16)
    return [{"x": xt[c]} for c in range(_NCORES)]


def kernel(x: np.ndarray) -> np.ndarray:
    from concourse.bass_utils import run_bass_kernel_spmd

    x = np.ascontiguousarray(np.asarray(x, dtype=np.float32))
    assert x.shape == (_B, _F, _D), x.shape

    nc = _get_nc()
    in_maps = _make_in_maps(x)
    res = run_bass_kernel_spmd(nc, in_maps, core_ids=list(range(_NCORES)))
    outs = []
    for c in range(_NCORES):
        o = res.results[c]["out"]  # [128, T]; o[p, t] = y[t*128 + p]
        outs.append(np.asarray(o).T.reshape(-1))
    return np.concatenate(outs).reshape(_B, 1).astype(np.float32)
